# revision 45
# baseline (speedup 1.0000x reference)
"""Trainium2 Bass kernel for nn_MultiHeadAttention (B=2, S=4096, D=512, H=8).

Computes: q/k/v = relu(x@W+b) per head, softmax(q k^T / sqrt(64)) v,
out = relu(concat_heads @ Wo + bo).

Sharding: 8 cores = 2 (batch) x 4 (query-slice).  Each core computes full
K/V projections for its batch (redundant across the 4 q-slice cores) and
attention + output projection for its 1024-row query slice.  No collectives;
the host concatenates the 8 output slices.

Per-core pipeline (vs the v1 baseline; ~372us -> ~308us):
- Projections (Q/K/V) run as fp8 DoubleRow matmuls (x and Wq/Wk/Wv in
  fp8e4, contracting 256 features per instruction); the output projection
  stays bf16 (fp8 there costs ~5x the final-error budget).  Bias+relu is
  fused into the PSUM->SBUF eviction (Q/V on ACT, K on DVE).
- Attention iterates ktiles: per ktile one pair of QK matmuls (the two
  heads of a pair sit in PE row groups 0-63/64-127 and run concurrently),
  one 1024-wide exp op over both heads' scores writes fp8 probabilities
  into an 8-slot SBUF ring, and (lagged 5 ktiles for pipelining) one fp8
  DoubleRow U matmul per head consumes two ring slots (contracting 256
  sequence positions per instruction).  V is stored fp8 with a ones
  column so U row 64 accumulates the softmax denominator for free.
- exp is split ~17/32 ACT (exact exp, fp8 out, bias folds the range
  shift) / 15/32 DVE (Schraudolph bit-trick: round(s*A+B) as int8 IS the
  fp8 of exp - one tensor_scalar op; ~2.7% rms which softmax
  normalization + the 2e-2 gate tolerate), so both engines compute exps
  concurrently with the PE.
- Denominator reciprocals run on ACT as exp(-ln(d)) over [1, 1024] rows
  (a patched table-set selection keeps exp/ln/relu in one ACT table so
  nothing reloads); broadcasts on gpsimd; normalize multiplies on DVE.
- Deferred projections and the first-half output projections fill PE
  bubbles inside exp-bound attention stretches; output rows overlap the
  final normalize via partially-open PSUM accumulation chains.
"""

import numpy as np
import ml_dtypes

import concourse.bass as bass
import concourse.mybir as mybir
import concourse.tile as tile
from concourse import bacc
from concourse import bass_utils
from concourse import hw_specs


def _patch_act_tables():
    """Make exp/relu/ln all resolve to the one table set that contains all
    three (natural_log_exp_and_others).  The load-insertion pass assigns
    each ACTIVATE the *first* set containing its function, so a kernel
    mixing exp and ln otherwise reloads tables around every ln (~2.7us per
    switch).  Only set *selection* changes; set contents seen by the
    runtime are untouched."""
    if getattr(hw_specs, "_mha_act_patch", False):
        return
    orig = hw_specs.get_activation_tables
    HOME = "natural_log_exp_and_others"
    AF_ = mybir.ActivationFunctionType

    def patched(arch):
        tables = orig(arch)
        if HOME not in tables:
            return tables
        out = {}
        for name, funcs in tables.items():
            if name != HOME:
                funcs = funcs - {AF_.Exp, AF_.Relu, AF_.Ln}
            out[name] = funcs
        return out

    hw_specs.get_activation_tables = patched
    bacc.get_activation_tables = patched
    hw_specs._mha_act_patch = True

F32 = mybir.dt.float32
BF16 = mybir.dt.bfloat16
FP8 = mybir.dt.float8e4
I8 = mybir.dt.int8
AF = mybir.ActivationFunctionType
ALU = mybir.AluOpType
DR = mybir.MatmulPerfMode.DoubleRow

P = 128
D = 512
H = 8
DH = 64
DT = D // P  # 4 (also = number of head pairs)
B = 2
S = 4096
NCORES = 8
QSPLIT = 4
SQ_FULL = S // QSPLIT  # 1024 query rows per core
QC = 512               # q-chunk (matmul free dim / PSUM bank width)
VP = 80                # padded V row stride (65 used; 80 keeps fp8 16B align)

# exp folding: pT = exp(s/8 + EXPB); the e^EXPB factor cancels in normalize.
EXPB = -2.9
LOG2E = 1.4426950408889634
# DVE bit-trick: int8(round(s*A8 + B8)) bits == fp8e4(exp(s/8 + EXPB))
A8 = (1 << 3) * LOG2E / 8.0
C8 = 0.35
B8 = 7 * (1 << 3) + (1 << 3) * LOG2E * EXPB - C8

# exp engine split: ACT_NUM of every 32 ktiles use ACT (exact exp);
# the rest use the DVE bit-trick.  Spread evenly (Bresenham).
ACT_NUM = 17


def exp_engine(kt, act_num=ACT_NUM):
    i = kt % 32
    return "a" if (i + 1) * act_num // 32 > i * act_num // 32 else "d"


def build_mha(sk=S, sq=SQ_FULL, skip_vbias=False):
    """Build the SPMD Bass program (identical on all cores).

    All inputs arrive pre-tiled by the host into exact SBUF layout
    ([128 partitions, contiguous free bytes]) so every load is a max-packet
    linear DMA."""
    _patch_act_tables()
    nc = bacc.Bacc("TRN2", target_bir_lowering=False, debug=False,
                   num_devices=NCORES)

    xT_d = nc.dram_tensor("xT_f8", (P, DT * sk), FP8,
                          kind="ExternalInput").ap()  # chunk-major, see prep
    xqT_d = nc.dram_tensor("xqT_f8", (P, DT * sq), FP8,
                           kind="ExternalInput").ap()
    w_dram = {}
    for n in ("wq", "wk", "wv"):
        w_dram[n] = nc.dram_tensor(n, (P, DT * D), FP8,
                                   kind="ExternalInput").ap()
    w_dram["wo"] = nc.dram_tensor("wo", (P, DT * D), BF16,
                                  kind="ExternalInput").ap()
    b_dram = {
        "bq": nc.dram_tensor("bq", (P, DT), F32, kind="ExternalInput").ap(),
        "bk": nc.dram_tensor("bk", (P, DT), F32, kind="ExternalInput").ap(),
        "bv": nc.dram_tensor("bv", (1, D), BF16, kind="ExternalInput").ap(),
        "bo": nc.dram_tensor("bo", (1, D), BF16, kind="ExternalInput").ap(),
    }
    out = nc.dram_tensor("out", (sq, D), F32, kind="ExternalOutput").ap()

    with tile.TileContext(nc) as tc:
        _build_tile(tc, xT_d, xqT_d, w_dram, b_dram, out, sk, sq,
                    skip_vbias)

    nc.compile()
    return nc


def _build_tile(tc, xT_d, xqT_d, w_dram, b_dram, out, sk, sq,
                skip_vbias=False):
    nc = tc.nc
    SK_T = sk // P            # ktiles of the key/value sequence (32)
    NKTP = SK_T // 2          # ktile pairs per head (16)
    SQ_T = sq // P
    NQC = sq // QC            # q chunks per core (2)
    CH = min(4, SK_T)         # stiles per projection chunk
    NCH = SK_T // CH

    with (
        tc.tile_pool(name="singles", bufs=1) as singles,
        tc.tile_pool(name="work", bufs=3) as work,
        tc.tile_pool(name="psum", bufs=2, space="PSUM") as psum,
    ):
        # ---- startup: only what Q-proj pair 0 needs, first ----
        w_bf = {}
        w_bf["wq"] = singles.tile([P, DT, D], FP8, name="wq_f8")
        wq_src = w_dram["wq"].rearrange("p (t n) -> p t n", t=DT)
        nc.sync.dma_start(w_bf["wq"][:, 0:2], wq_src[:, 0:2])
        xTq = singles.tile([P, DT, sq], FP8)
        xTq_src = xqT_d.rearrange("p (t s) -> p t s", t=DT)
        nc.scalar.dma_start(xTq[:, 0:2], xTq_src[:, 0:2])
        b_col = {}
        b_col["bq"] = singles.tile([P, DT], F32, name="bq_col")
        nc.sync.dma_start(w_bf["wq"][:, 2:4], wq_src[:, 2:4])
        nc.scalar.dma_start(xTq[:, 2:4], xTq_src[:, 2:4])
        nc.scalar.dma_start(b_col["bq"], b_dram["bq"])

        QT = singles.tile([P, DT, sq], BF16)

        def qproj(j, nq):
            psQ = psum.tile([P, QC], F32, tag="proj", name="psQ")
            for t2 in range(DT // 2):
                nc.tensor.matmul(
                    psQ, w_bf["wq"][:, 2 * t2:2 * t2 + 2, j * P:(j + 1) * P],
                    xTq[:, 2 * t2:2 * t2 + 2, nq * QC:(nq + 1) * QC],
                    start=(t2 == 0), stop=(t2 == DT // 2 - 1),
                    perf_mode=DR)
            nc.scalar.activation(
                QT[:, j, nq * QC:(nq + 1) * QC], psQ, AF.Relu,
                bias=b_col["bq"][:, j:j + 1])

        qproj(0, 0)
        if NQC > 1:
            qproj(0, 1)

        # ---- K-proj deps next (attention can start before V exists) ----
        b_row = {}
        w_bf["wk"] = singles.tile([P, DT, D], FP8, name="wk_f8")
        nc.scalar.dma_start(w_bf["wk"], w_dram["wk"].rearrange(
            "p (t n) -> p t n", t=DT))
        b_col["bk"] = singles.tile([P, DT], F32, name="bk_col")
        nc.scalar.dma_start(b_col["bk"], b_dram["bk"])
        CHP = CH * P
        xT = singles.tile([P, NCH, DT, CHP], FP8)
        xT_src = xT_d.rearrange("p (n t s) -> p n t s", n=NCH, t=DT)
        nc.sync.dma_start(xT[:, 0], xT_src[:, 0])
        for n in ("wv", "wo"):
            dt_n = BF16 if n == "wo" else FP8
            wb = singles.tile([P, DT, D], dt_n, name=f"{n}_w")
            nc.sync.dma_start(wb, w_dram[n].rearrange(
                "p (t n) -> p t n", t=DT))
            w_bf[n] = wb
            if n == "wv" and not skip_vbias:
                br = singles.tile([1, D], BF16, name="bv_row")
                nc.sync.dma_start(br, b_dram["bv"])
                b_row["bv"] = br
        br = singles.tile([1, D], BF16, name="bo_row")
        nc.sync.dma_start(br, b_dram["bo"])
        b_row["bo"] = br

        # ---- persistent SBUF tensors ----
        bias_t = singles.tile([P, 1], F32)
        nc.vector.memset(bias_t, EXPB)
        xT1 = None
        if not skip_vbias:
            xT1 = singles.tile([1, sk], BF16)
            nc.vector.memset(xT1, 1.0)
        KT = singles.tile([P, DT, sk], BF16)
        V_pad = singles.tile([P, NKTP, H, 2, VP], FP8)
        nc.vector.memset(V_pad[:, :, :, :, DH:DH + 1], 1.0)
        OT = singles.tile([P, DT, sq], BF16)
        OT1 = singles.tile([1, sq], BF16)
        nc.vector.memset(OT1, 1.0)

        # PSUM tags: "proj" 2x1 banks, "scores" 2x2 banks, "psU" 1x2 = 8
        def vproj(st):
            n, si = st // CH, st % CH
            psV = psum.tile([P, D], F32, tag="proj", name="psV")
            for t2 in range(DT // 2):
                nc.tensor.matmul(
                    psV, xT[:, n, 2 * t2:2 * t2 + 2, si * P:(si + 1) * P],
                    w_bf["wv"][:, 2 * t2:2 * t2 + 2, :],
                    start=(t2 == 0),
                    stop=(skip_vbias and t2 == DT // 2 - 1),
                    perf_mode=DR)
            if not skip_vbias:
                nc.tensor.matmul(psV, xT1[:, st * P:(st + 1) * P],
                                 b_row["bv"], start=False, stop=True)
            nc.scalar.activation(
                V_pad[:, st // 2, :, st % 2, 0:DH],
                psV.rearrange("p (h d) -> p h d", h=H), AF.Relu)

        def kproj(j, n):
            psK = psum.tile([P, CH * P], F32, tag="proj", name="psK")
            for t2 in range(DT // 2):
                nc.tensor.matmul(
                    psK, w_bf["wk"][:, 2 * t2:2 * t2 + 2, j * P:(j + 1) * P],
                    xT[:, n, 2 * t2:2 * t2 + 2, :],
                    start=(t2 == 0), stop=(t2 == DT // 2 - 1),
                    perf_mode=DR)
            nc.vector.tensor_scalar(
                KT[:, j, n * CH * P:(n + 1) * CH * P], psK,
                b_col["bk"][:, j:j + 1], 0.0, op0=ALU.add, op1=ALU.max)

        # fp8 probability ring: slot kt%RING holds exp'd scores for both
        # heads of one ktile; the U matmul reads two adjacent slots with a
        # strided DoubleRow access pattern.
        RING = 8
        PT = singles.tile([P, RING, 2, QC], FP8, name="PT_ring")

        def qk1(j, qc, kt, eng):
            """Scores + exp for BOTH heads of pair j at ktile kt.  The two
            QK matmuls sit in different PE row groups (partitions 0-63 vs
            64-127) and run concurrently; one 1024-wide exp op (eng 'a' =
            ACT exact exp->fp8, 'd' = DVE bit-trick int8-as-fp8) covers
            both heads."""
            q0 = qc * QC
            psS = psum.tile([P, 2, QC], F32, tag="scores", bufs=2,
                            name="psS")
            for a in (0, 1):
                h0 = a * DH
                nc.tensor.matmul(
                    psS[:, a, :],
                    KT[h0:h0 + DH, j, kt * P:(kt + 1) * P],
                    QT[h0:h0 + DH, j, q0:q0 + QC], start=True, stop=True)
            slot = kt % RING
            pT_f = PT[:, slot].rearrange("p a b -> p (a b)")
            psS_f = psS.rearrange("p a b -> p (a b)")
            if eng == "a":
                nc.scalar.activation(pT_f, psS_f, AF.Exp, scale=0.125,
                                     bias=bias_t)
            else:
                nc.vector.tensor_scalar(pT_f.bitcast(I8), psS_f, A8, B8,
                                        op0=ALU.mult, op1=ALU.add)

        def u_pair(j, tp, psU):
            """DoubleRow U matmuls for both heads of ktile pair tp, reading
            ring slots (2tp)%RING, (2tp)%RING+1 (slot stride 2*QC fp8)."""
            s0 = (2 * tp) % RING
            for a in (0, 1):
                nc.tensor.matmul(
                    psU[:, a, :], V_pad[:, tp, 2 * j + a, :, 0:DH + 1],
                    PT[:, s0:s0 + 2, a, :],
                    start=(tp == 0), stop=(tp == NKTP - 1), perf_mode=DR)

        brc_sink = {}

        def finish_block(j, qc, psU):
            """U done for both heads: copy U rows out of PSUM, compute
            1/denominator on ACT (exp(-ln d)), then normalize on gpsimd."""
            q0 = qc * QC
            ucs = work.tile([DH, 2, QC], F32, tag="ucopy", bufs=2,
                            name="ucs")
            nc.vector.tensor_copy(ucs, psU[0:DH])
            # Ln reads the denominator row at partition 64 and lands it at
            # partition 0 (ACT maps partitions relative to the AP base)
            lnd = work.tile([1, 2 * QC], F32, tag="lnd", bufs=2, name="lnd")
            nc.scalar.activation(
                lnd, psU[DH:DH + 1].rearrange("p a b -> p (a b)"), AF.Ln)
            rcp = work.tile([1, 2 * QC], F32, tag="rcp", bufs=2, name="rcp")
            nc.scalar.activation(rcp, lnd, AF.Exp, scale=-1.0)
            for a in (0, 1):
                h0 = a * DH
                brc = work.tile([DH, QC], F32, tag="brc", bufs=4,
                                name="brc")
                nc.gpsimd.partition_broadcast(
                    brc, rcp[0:1, a * QC:a * QC + QC])
                nc.vector.tensor_mul(
                    OT[h0:h0 + DH, j, q0:q0 + QC], ucs[:, a, :], brc)
                brc_sink[(j, qc)] = brc

        def attn_span(j, qc, kts, psU, fillers=(), drain_dve=False,
                      act_num=ACT_NUM):
            """Emit one attention block: per ktile a QK pair + exp
            (engines alternating by ktile), with the U matmul pair lagging
            two ktiles behind so the in-order PE never waits on an exp.
            Fillers (deferred projections) slot in between ktiles."""
            fillers = list(fillers)
            spacing = max(1, (3 * len(kts) // 4) // (len(fillers) + 1))
            for i, kt in enumerate(kts):
                if kt >= 5 and kt % 2 == 1:
                    u_pair(j, (kt - 5) // 2, psU)
                eng = "a" if (drain_dve and kt >= SK_T - 6) \
                    else exp_engine(kt, act_num)
                qk1(j, qc, kt, eng)
                if fillers and (i + 1) % spacing == 0:
                    fillers.pop(0)()
            for f in fillers:
                f()
            if kts[-1] == SK_T - 1:
                u_pair(j, NKTP - 2, psU)
                u_pair(j, NKTP - 1, psU)
                finish_block(j, qc, psU)

        def new_psU():
            return psum.tile([DH + 1, 2, QC], F32, tag="psU", bufs=1,
                             name="psU")

        def outproj(qt):
            # bias matmul first: it reads OT1, whose re-write after the last
            # normalize acts as a scheduling gate for the whole chain
            psO = psum.tile([P, D], F32, tag="proj", name="psO")
            nc.tensor.matmul(psO, OT1[:, qt * P:(qt + 1) * P],
                             b_row["bo"], start=True, stop=False)
            for j in range(DT):
                nc.tensor.matmul(psO, OT[:, j, qt * P:(qt + 1) * P],
                                 w_bf["wo"][:, j, :],
                                 start=False, stop=(j == DT - 1))
            o_sb = work.tile([P, D], F32, tag="osb", bufs=2, name="o_sb")
            nc.scalar.activation(o_sb, psO, AF.Relu)
            nc.sync.dma_start(out[qt * P:(qt + 1) * P, :], o_sb)

        def gate_outproj(blk):
            """No-op rewrite of OT1 (max(1, rcp<1) == 1) that depends on
            block `blk`'s normalize chain — gates the outproj chains (which
            start with an OT1-reading bias matmul) behind it."""
            brc = brc_sink[blk]
            nc.vector.tensor_scalar(OT1, OT1, brc[0:1, 0:1], None,
                                    op0=ALU.max)

        # ---- chunk loop: x load + V proj + K proj(pair 0) + attn(0, 0);
        # exps overlap the vproj matmuls, U runs after its V is written ----
        psU0 = new_psU()
        kproj(0, 0)
        for n in range(NCH):
            if n + 1 < NCH:
                nc.sync.dma_start(xT[:, n + 1], xT_src[:, n + 1])
            for i in range(CH // 2):
                kt0 = n * CH + 2 * i
                qk1(0, 0, kt0, exp_engine(kt0))
                qk1(0, 0, kt0 + 1, exp_engine(kt0 + 1))
                vproj(kt0)
                vproj(kt0 + 1)
                if i == 0 and n + 1 < NCH:
                    kproj(0, n + 1)
            for i in range(CH // 2):
                u_pair(0, n * CH // 2 + i, psU0)
            if (n + 1) * CH == SK_T:
                finish_block(0, 0, psU0)

        # ---- remaining blocks, qc-major; fillers carry the next block's
        # projections plus the first-half output projections ----
        blocks = [(j, 0) for j in range(1, DT)]
        blocks += [(j, 1) for j in range(DT)] if NQC > 1 else []
        owed = {blk: [] for blk in blocks}
        for (j, qc) in blocks:
            if not (j == 0 and qc <= 1):
                owed[(j, qc)].append(lambda j=j, qc=qc: qproj(j, qc))
            if qc == 0 and j >= 1:
                for n in range(NCH):
                    owed[(j, qc)].append(lambda j=j, n=n: kproj(j, n))
        # first-half outproj: OT rows for qc=0 complete after block (DT-1, 0);
        # run them inside the following blocks
        if NQC > 1:
            mid_i = blocks.index((0, 1))
            later = blocks[mid_i + 1]
            owed[later].append(lambda: gate_outproj((DT - 1, 0)))
            half = SQ_T // NQC
            for qt in range(half // 2):
                owed[later].append(lambda qt=qt: outproj(qt))
            for qt in range(half // 2, half):
                owed[blocks[mid_i + 2]].append(lambda qt=qt: outproj(qt))

        qt_lo = SQ_T // NQC if NQC > 1 else 0
        open_psO = []

        def open_chain(qt):
            """Partial outproj chain (bias + first DT-1 weight tiles): its
            OT inputs are ready before the last block, so it can fill the
            last block's PE bubbles; the final tile waits the last
            normalize."""
            psO = psum.tile([P, D], F32, tag="proj", name="psO")
            nc.tensor.matmul(psO, OT1[:, qt * P:(qt + 1) * P],
                             b_row["bo"], start=True, stop=False)
            for j in range(DT - 1):
                nc.tensor.matmul(psO, OT[:, j, qt * P:(qt + 1) * P],
                                 w_bf["wo"][:, j, :],
                                 start=False, stop=False)
            open_psO.append((qt, psO))

        last_fillers = [lambda: open_chain(qt_lo),
                        lambda: open_chain(qt_lo + 1)]

        for f in owed[blocks[0]]:
            f()
        for bi, (j, qc) in enumerate(blocks):
            fillers = []
            if bi + 1 < len(blocks):
                fillers += owed[blocks[bi + 1]]
            else:
                fillers += last_fillers
            psU = new_psU()
            attn_span(j, qc, list(range(SK_T)), psU, fillers,
                      drain_dve=(bi == len(blocks) - 1),
                      act_num=ACT_NUM)

        # ---- tail: last block's normalize + remaining output rows;
        # the two partially-open chains were emitted inside the last block.
        gate_outproj(blocks[-1])
        for qt, psO in open_psO:
            nc.tensor.matmul(psO, OT[:, DT - 1, qt * P:(qt + 1) * P],
                             w_bf["wo"][:, DT - 1, :],
                             start=False, stop=True)
            o_sb = work.tile([P, D], F32, tag="osb", bufs=2, name="o_sb")
            nc.scalar.activation(o_sb, psO, AF.Relu)
            nc.sync.dma_start(out[qt * P:(qt + 1) * P, :], o_sb)
        for qt in range(qt_lo + 2, SQ_T):
            outproj(qt)


_NC_CACHE = {}


def _get_nc(sk=S, sq=SQ_FULL, skip_vbias=False):
    key = (sk, sq, skip_vbias)
    if key not in _NC_CACHE:
        _NC_CACHE[key] = build_mha(sk, sq, skip_vbias)
    return _NC_CACHE[key]


def _tile_rows(a):
    """[D, n] -> SBUF layout [P, DT*n]: partition p gets rows p, 128+p, ..."""
    Dd, n = a.shape
    t = Dd // P
    return np.ascontiguousarray(
        a.reshape(t, P, n).transpose(1, 0, 2).reshape(P, t * n))


def _tile_chunks(a, chp):
    """[D, sk] -> chunk-major SBUF layout [P, NCH*DT*chp]: per partition,
    sequence chunks outermost so each chunk is one contiguous linear DMA."""
    Dd, sk = a.shape
    t, nch = Dd // P, sk // chp
    return np.ascontiguousarray(
        a.reshape(t, P, nch, chp).transpose(1, 2, 0, 3).reshape(P, -1))


def prep_inputs(x, Wq, bq, Wk, bk, Wv, bv, Wo, bo):
    """Host-side sharding/layout prep: bf16 casts, feature-major transpose,
    SBUF pre-tiling.  Returns the 8 per-core input maps."""
    bf = ml_dtypes.bfloat16
    f8 = ml_dtypes.float8_e4m3
    x = np.asarray(x, dtype=np.float32)
    shared = {
        "wq": _tile_rows(np.asarray(Wq, np.float32).astype(f8)),
        "wk": _tile_rows(np.asarray(Wk, np.float32).astype(f8)),
        "wv": _tile_rows(np.asarray(Wv, np.float32).astype(f8)),
        "wo": _tile_rows(np.asarray(Wo, np.float32).astype(bf)),
        "bq": np.ascontiguousarray(
            np.asarray(bq, np.float32).reshape(DT, P).T),
        "bk": np.ascontiguousarray(
            np.asarray(bk, np.float32).reshape(DT, P).T),
        "bv": np.asarray(bv, np.float32).astype(bf).reshape(1, D),
        "bo": np.asarray(bo, np.float32).astype(bf).reshape(1, D),
    }
    xT_b = [x[b].T.astype(f8) for b in range(B)]
    xT_tiled = [_tile_chunks(xb, 4 * P) for xb in xT_b]
    in_maps = []
    for c in range(NCORES):
        b, qo = divmod(c, QSPLIT)
        m = dict(shared)
        m["xT_f8"] = xT_tiled[b]
        m["xqT_f8"] = _tile_rows(
            xT_b[b][:, qo * SQ_FULL:(qo + 1) * SQ_FULL])
        in_maps.append(m)
    return in_maps


def kernel(x, Wq, bq, Wk, bk, Wv, bv, Wo, bo, **run_kwargs):
    """Full-input entry point: shards across 8 NeuronCores, returns full out."""
    in_maps = prep_inputs(x, Wq, bq, Wk, bk, Wv, bv, Wo, bo)
    nc = _get_nc(skip_vbias=bool(np.all(np.asarray(bv) == 0)))
    res = bass_utils.run_bass_kernel_spmd(
        nc, in_maps, core_ids=list(range(NCORES)), **run_kwargs)
    full = np.empty((B, S, D), np.float32)
    for c in range(NCORES):
        b, qo = divmod(c, QSPLIT)
        full[b, qo * SQ_FULL:(qo + 1) * SQ_FULL] = res.results[c]["out"]
    if run_kwargs:
        return full, res
    return full


# revision 48
# speedup vs baseline: 1.0090x; 1.0090x over previous
"""Trainium2 Bass kernel for nn_MultiHeadAttention (B=2, S=4096, D=512, H=8).

Computes: q/k/v = relu(x@W+b) per head, softmax(q k^T / sqrt(64)) v,
out = relu(concat_heads @ Wo + bo).

Sharding: 8 cores = 2 (batch) x 4 (query-slice).  Each core computes full
K/V projections for its batch (redundant across the 4 q-slice cores) and
attention + output projection for its 1024-row query slice.  No collectives;
the host concatenates the 8 output slices.

Per-core pipeline (vs the v1 baseline; ~372us -> ~308us):
- Projections (Q/K/V) run as fp8 DoubleRow matmuls (x and Wq/Wk/Wv in
  fp8e4, contracting 256 features per instruction); the output projection
  stays bf16 (fp8 there costs ~5x the final-error budget).  Bias+relu is
  fused into the PSUM->SBUF eviction (Q/V on ACT, K on DVE).
- Attention iterates ktiles: per ktile one pair of QK matmuls (the two
  heads of a pair sit in PE row groups 0-63/64-127 and run concurrently),
  one 1024-wide exp op over both heads' scores writes fp8 probabilities
  into an 8-slot SBUF ring, and (lagged 5 ktiles for pipelining) one fp8
  DoubleRow U matmul per head consumes two ring slots (contracting 256
  sequence positions per instruction).  V is stored fp8 with a ones
  column so U row 64 accumulates the softmax denominator for free.
- exp is split ~17/32 ACT (exact exp, fp8 out, bias folds the range
  shift) / 15/32 DVE (Schraudolph bit-trick: round(s*A+B) as int8 IS the
  fp8 of exp - one tensor_scalar op; ~2.7% rms which softmax
  normalization + the 2e-2 gate tolerate), so both engines compute exps
  concurrently with the PE.
- Denominator reciprocals run on ACT as exp(-ln(d)) over [1, 1024] rows
  (a patched table-set selection keeps exp/ln/relu in one ACT table so
  nothing reloads); broadcasts on gpsimd; normalize multiplies on DVE.
- Deferred projections and the first-half output projections fill PE
  bubbles inside exp-bound attention stretches; output rows overlap the
  final normalize via partially-open PSUM accumulation chains.
"""

import numpy as np
import ml_dtypes

import concourse.bass as bass
import concourse.mybir as mybir
import concourse.tile as tile
from concourse import bacc
from concourse import bass_utils
from concourse import hw_specs


def _patch_act_tables():
    """Make exp/relu/ln all resolve to the one table set that contains all
    three (natural_log_exp_and_others).  The load-insertion pass assigns
    each ACTIVATE the *first* set containing its function, so a kernel
    mixing exp and ln otherwise reloads tables around every ln (~2.7us per
    switch).  Only set *selection* changes; set contents seen by the
    runtime are untouched."""
    if getattr(hw_specs, "_mha_act_patch", False):
        return
    orig = hw_specs.get_activation_tables
    HOME = "natural_log_exp_and_others"
    AF_ = mybir.ActivationFunctionType

    def patched(arch):
        tables = orig(arch)
        if HOME not in tables:
            return tables
        out = {}
        for name, funcs in tables.items():
            if name != HOME:
                funcs = funcs - {AF_.Exp, AF_.Relu, AF_.Ln}
            out[name] = funcs
        return out

    hw_specs.get_activation_tables = patched
    bacc.get_activation_tables = patched
    hw_specs._mha_act_patch = True

F32 = mybir.dt.float32
BF16 = mybir.dt.bfloat16
FP8 = mybir.dt.float8e4
I8 = mybir.dt.int8
AF = mybir.ActivationFunctionType
ALU = mybir.AluOpType
DR = mybir.MatmulPerfMode.DoubleRow

P = 128
D = 512
H = 8
DH = 64
DT = D // P  # 4 (also = number of head pairs)
B = 2
S = 4096
NCORES = 8
QSPLIT = 4
SQ_FULL = S // QSPLIT  # 1024 query rows per core
QC = 512               # q-chunk (matmul free dim / PSUM bank width)
VP = 80                # padded V row stride (65 used; 80 keeps fp8 16B align)

# exp folding: pT = exp(s/8 + EXPB); the e^EXPB factor cancels in normalize.
EXPB = -2.9
LOG2E = 1.4426950408889634
# DVE bit-trick: int8(round(s*A8 + B8)) bits == fp8e4(exp(s/8 + EXPB))
A8 = (1 << 3) * LOG2E / 8.0
C8 = 0.35
B8 = 7 * (1 << 3) + (1 << 3) * LOG2E * EXPB - C8

# exp engine split: ACT_NUM of every 32 ktiles use ACT (exact exp);
# the rest use the DVE bit-trick.  Spread evenly (Bresenham).
ACT_NUM = 17


def exp_engine(kt, act_num=ACT_NUM):
    i = kt % 32
    return "a" if (i + 1) * act_num // 32 > i * act_num // 32 else "d"


def build_mha(sk=S, sq=SQ_FULL, skip_vbias=False, skip_obias=False):
    """Build the SPMD Bass program (identical on all cores).

    All inputs arrive pre-tiled by the host into exact SBUF layout
    ([128 partitions, contiguous free bytes]) so every load is a max-packet
    linear DMA."""
    _patch_act_tables()
    nc = bacc.Bacc("TRN2", target_bir_lowering=False, debug=False,
                   num_devices=NCORES)

    xT_d = nc.dram_tensor("xT_f8", (P, DT * sk), FP8,
                          kind="ExternalInput").ap()  # chunk-major, see prep
    xqT_d = nc.dram_tensor("xqT_f8", (P, DT * sq), FP8,
                           kind="ExternalInput").ap()
    w_dram = {}
    for n in ("wq", "wk", "wv"):
        w_dram[n] = nc.dram_tensor(n, (P, DT * D), FP8,
                                   kind="ExternalInput").ap()
    w_dram["wo"] = nc.dram_tensor("wo", (P, DT * D), BF16,
                                  kind="ExternalInput").ap()
    b_dram = {
        "bq": nc.dram_tensor("bq", (P, DT), F32, kind="ExternalInput").ap(),
        "bk": nc.dram_tensor("bk", (P, DT), F32, kind="ExternalInput").ap(),
        "bv": nc.dram_tensor("bv", (1, D), BF16, kind="ExternalInput").ap(),
        "bo": nc.dram_tensor("bo", (1, D), BF16, kind="ExternalInput").ap(),
    }
    out = nc.dram_tensor("out", (sq, D), F32, kind="ExternalOutput").ap()

    with tile.TileContext(nc) as tc:
        _build_tile(tc, xT_d, xqT_d, w_dram, b_dram, out, sk, sq,
                    skip_vbias, skip_obias)

    nc.compile()
    return nc


def _build_tile(tc, xT_d, xqT_d, w_dram, b_dram, out, sk, sq,
                skip_vbias=False, skip_obias=False):
    nc = tc.nc
    SK_T = sk // P            # ktiles of the key/value sequence (32)
    NKTP = SK_T // 2          # ktile pairs per head (16)
    SQ_T = sq // P
    NQC = sq // QC            # q chunks per core (2)
    CH = min(4, SK_T)         # stiles per projection chunk
    NCH = SK_T // CH

    with (
        tc.tile_pool(name="singles", bufs=1) as singles,
        tc.tile_pool(name="work", bufs=3) as work,
        tc.tile_pool(name="psum", bufs=2, space="PSUM") as psum,
    ):
        # ---- startup: only what Q-proj pair 0 needs, first ----
        w_bf = {}
        w_bf["wq"] = singles.tile([P, DT, D], FP8, name="wq_f8")
        wq_src = w_dram["wq"].rearrange("p (t n) -> p t n", t=DT)
        nc.sync.dma_start(w_bf["wq"][:, 0:2], wq_src[:, 0:2])
        xTq = singles.tile([P, DT, sq], FP8)
        xTq_src = xqT_d.rearrange("p (t s) -> p t s", t=DT)
        nc.scalar.dma_start(xTq[:, 0:2], xTq_src[:, 0:2])
        b_col = {}
        b_col["bq"] = singles.tile([P, DT], F32, name="bq_col")
        nc.sync.dma_start(w_bf["wq"][:, 2:4], wq_src[:, 2:4])
        nc.scalar.dma_start(xTq[:, 2:4], xTq_src[:, 2:4])
        nc.scalar.dma_start(b_col["bq"], b_dram["bq"])

        QT = singles.tile([P, DT, sq], BF16)

        def qproj(j, nq):
            psQ = psum.tile([P, QC], F32, tag="proj", name="psQ")
            for t2 in range(DT // 2):
                nc.tensor.matmul(
                    psQ, w_bf["wq"][:, 2 * t2:2 * t2 + 2, j * P:(j + 1) * P],
                    xTq[:, 2 * t2:2 * t2 + 2, nq * QC:(nq + 1) * QC],
                    start=(t2 == 0), stop=(t2 == DT // 2 - 1),
                    perf_mode=DR)
            nc.scalar.activation(
                QT[:, j, nq * QC:(nq + 1) * QC], psQ, AF.Relu,
                bias=b_col["bq"][:, j:j + 1])

        qproj(0, 0)
        if NQC > 1:
            qproj(0, 1)

        # ---- K-proj deps next (attention can start before V exists) ----
        b_row = {}
        w_bf["wk"] = singles.tile([P, DT, D], FP8, name="wk_f8")
        nc.scalar.dma_start(w_bf["wk"], w_dram["wk"].rearrange(
            "p (t n) -> p t n", t=DT))
        b_col["bk"] = singles.tile([P, DT], F32, name="bk_col")
        nc.scalar.dma_start(b_col["bk"], b_dram["bk"])
        CHP = CH * P
        xT = singles.tile([P, NCH, DT, CHP], FP8)
        xT_src = xT_d.rearrange("p (n t s) -> p n t s", n=NCH, t=DT)
        nc.sync.dma_start(xT[:, 0], xT_src[:, 0])
        for n in ("wv", "wo"):
            dt_n = BF16 if n == "wo" else FP8
            wb = singles.tile([P, DT, D], dt_n, name=f"{n}_w")
            nc.sync.dma_start(wb, w_dram[n].rearrange(
                "p (t n) -> p t n", t=DT))
            w_bf[n] = wb
            if n == "wv" and not skip_vbias:
                br = singles.tile([1, D], BF16, name="bv_row")
                nc.sync.dma_start(br, b_dram["bv"])
                b_row["bv"] = br
        br = singles.tile([1, D], BF16, name="bo_row")
        nc.sync.dma_start(br, b_dram["bo"])
        b_row["bo"] = br

        # ---- persistent SBUF tensors ----
        bias_t = singles.tile([P, 1], F32)
        nc.vector.memset(bias_t, EXPB)
        xT1 = None
        if not skip_vbias:
            xT1 = singles.tile([1, sk], BF16)
            nc.vector.memset(xT1, 1.0)
        KT = singles.tile([P, DT, sk], BF16)
        V_pad = singles.tile([P, NKTP, H, 2, VP], FP8)
        nc.vector.memset(V_pad[:, :, :, :, DH:DH + 1], 1.0)
        OT = singles.tile([P, DT, sq], BF16)
        OT1 = singles.tile([1, sq], BF16)
        nc.vector.memset(OT1, 1.0)

        # PSUM tags: "proj" 2x1 banks, "scores" 2x2 banks, "psU" 1x2 = 8
        def vproj(st):
            n, si = st // CH, st % CH
            psV = psum.tile([P, D], F32, tag="proj", name="psV")
            for t2 in range(DT // 2):
                nc.tensor.matmul(
                    psV, xT[:, n, 2 * t2:2 * t2 + 2, si * P:(si + 1) * P],
                    w_bf["wv"][:, 2 * t2:2 * t2 + 2, :],
                    start=(t2 == 0),
                    stop=(skip_vbias and t2 == DT // 2 - 1),
                    perf_mode=DR)
            if not skip_vbias:
                nc.tensor.matmul(psV, xT1[:, st * P:(st + 1) * P],
                                 b_row["bv"], start=False, stop=True)
            nc.scalar.activation(
                V_pad[:, st // 2, :, st % 2, 0:DH],
                psV.rearrange("p (h d) -> p h d", h=H), AF.Relu)

        def kproj(j, n):
            psK = psum.tile([P, CH * P], F32, tag="proj", name="psK")
            for t2 in range(DT // 2):
                nc.tensor.matmul(
                    psK, w_bf["wk"][:, 2 * t2:2 * t2 + 2, j * P:(j + 1) * P],
                    xT[:, n, 2 * t2:2 * t2 + 2, :],
                    start=(t2 == 0), stop=(t2 == DT // 2 - 1),
                    perf_mode=DR)
            nc.vector.tensor_scalar(
                KT[:, j, n * CH * P:(n + 1) * CH * P], psK,
                b_col["bk"][:, j:j + 1], 0.0, op0=ALU.add, op1=ALU.max)

        # fp8 probability ring: slot kt%RING holds exp'd scores for both
        # heads of one ktile; the U matmul reads two adjacent slots with a
        # strided DoubleRow access pattern.
        RING = 8
        PT = singles.tile([P, RING, 2, QC], FP8, name="PT_ring")

        def qk1(j, qc, kt, eng):
            """Scores + exp for BOTH heads of pair j at ktile kt.  The two
            QK matmuls sit in different PE row groups (partitions 0-63 vs
            64-127) and run concurrently; one 1024-wide exp op (eng 'a' =
            ACT exact exp->fp8, 'd' = DVE bit-trick int8-as-fp8) covers
            both heads."""
            q0 = qc * QC
            psS = psum.tile([P, 2, QC], F32, tag="scores", bufs=2,
                            name="psS")
            for a in (0, 1):
                h0 = a * DH
                nc.tensor.matmul(
                    psS[:, a, :],
                    KT[h0:h0 + DH, j, kt * P:(kt + 1) * P],
                    QT[h0:h0 + DH, j, q0:q0 + QC], start=True, stop=True)
            slot = kt % RING
            pT_f = PT[:, slot].rearrange("p a b -> p (a b)")
            psS_f = psS.rearrange("p a b -> p (a b)")
            if eng == "a":
                nc.scalar.activation(pT_f, psS_f, AF.Exp, scale=0.125,
                                     bias=bias_t)
            else:
                nc.vector.tensor_scalar(pT_f.bitcast(I8), psS_f, A8, B8,
                                        op0=ALU.mult, op1=ALU.add)

        def u_pair(j, tp, psU):
            """DoubleRow U matmuls for both heads of ktile pair tp, reading
            ring slots (2tp)%RING, (2tp)%RING+1 (slot stride 2*QC fp8)."""
            s0 = (2 * tp) % RING
            for a in (0, 1):
                nc.tensor.matmul(
                    psU[:, a, :], V_pad[:, tp, 2 * j + a, :, 0:DH + 1],
                    PT[:, s0:s0 + 2, a, :],
                    start=(tp == 0), stop=(tp == NKTP - 1), perf_mode=DR)

        brc_sink = {}

        def finish_block(j, qc, psU):
            """U done for both heads: copy U rows out of PSUM, compute
            1/denominator on ACT (exp(-ln d)), then normalize on gpsimd."""
            q0 = qc * QC
            ucs = work.tile([DH, 2, QC], F32, tag="ucopy", bufs=2,
                            name="ucs")
            nc.vector.tensor_copy(ucs, psU[0:DH])
            # Ln reads the denominator row at partition 64 and lands it at
            # partition 0 (ACT maps partitions relative to the AP base)
            lnd = work.tile([1, 2 * QC], F32, tag="lnd", bufs=2, name="lnd")
            nc.scalar.activation(
                lnd, psU[DH:DH + 1].rearrange("p a b -> p (a b)"), AF.Ln)
            rcp = work.tile([1, 2 * QC], F32, tag="rcp", bufs=2, name="rcp")
            nc.scalar.activation(rcp, lnd, AF.Exp, scale=-1.0)
            for a in (0, 1):
                h0 = a * DH
                brc = work.tile([DH, QC], F32, tag="brc", bufs=4,
                                name="brc")
                nc.gpsimd.partition_broadcast(
                    brc, rcp[0:1, a * QC:a * QC + QC])
                nc.vector.tensor_mul(
                    OT[h0:h0 + DH, j, q0:q0 + QC], ucs[:, a, :], brc)
                brc_sink[(j, qc)] = brc

        def attn_span(j, qc, kts, psU, fillers=(), drain_dve=False,
                      act_num=ACT_NUM):
            """Emit one attention block: per ktile a QK pair + exp
            (engines alternating by ktile), with the U matmul pair lagging
            two ktiles behind so the in-order PE never waits on an exp.
            Fillers (deferred projections) slot in between ktiles."""
            fillers = list(fillers)
            spacing = max(1, (3 * len(kts) // 4) // (len(fillers) + 1))
            for i, kt in enumerate(kts):
                if kt >= 5 and kt % 2 == 1:
                    u_pair(j, (kt - 5) // 2, psU)
                eng = "a" if (drain_dve and kt >= SK_T - 6) \
                    else exp_engine(kt, act_num)
                qk1(j, qc, kt, eng)
                if fillers and (i + 1) % spacing == 0:
                    fillers.pop(0)()
            for f in fillers:
                f()
            if kts[-1] == SK_T - 1:
                u_pair(j, NKTP - 2, psU)
                u_pair(j, NKTP - 1, psU)
                finish_block(j, qc, psU)

        def new_psU():
            return psum.tile([DH + 1, 2, QC], F32, tag="psU", bufs=1,
                             name="psU")

        def obias_mm(psO, qt):
            """Ordering gate (+ bias when bo!=0): reads OT1, whose re-write
            after the last normalize gates the chain.  With bo==0 only the
            dependency matters, so stream a single column (~60 cycles); the
            j=0 weight matmul then clears PSUM with start=True."""
            if skip_obias:
                nc.tensor.matmul(psO[:, 0:1], OT1[:, qt * P:(qt + 1) * P],
                                 b_row["bo"][:, 0:1], start=True, stop=False,
                                 skip_group_check=True)
                return True
            nc.tensor.matmul(psO, OT1[:, qt * P:(qt + 1) * P],
                             b_row["bo"], start=True, stop=False)
            return False

        def outproj(qt):
            psO = psum.tile([P, D], F32, tag="proj", name="psO")
            restart = obias_mm(psO, qt)
            for j in range(DT):
                nc.tensor.matmul(psO, OT[:, j, qt * P:(qt + 1) * P],
                                 w_bf["wo"][:, j, :],
                                 start=(restart and j == 0),
                                 stop=(j == DT - 1))
            o_sb = work.tile([P, D], F32, tag="osb", bufs=2, name="o_sb")
            nc.scalar.activation(o_sb, psO, AF.Relu)
            nc.sync.dma_start(out[qt * P:(qt + 1) * P, :], o_sb)

        def gate_outproj(blk):
            """No-op rewrite of OT1 (max(1, rcp<1) == 1) that depends on
            block `blk`'s normalize chain — gates the outproj chains (which
            start with an OT1-reading bias matmul) behind it."""
            brc = brc_sink[blk]
            nc.vector.tensor_scalar(OT1, OT1, brc[0:1, 0:1], None,
                                    op0=ALU.max)

        # ---- chunk loop: x load + V proj + K proj(pair 0) + attn(0, 0);
        # exps overlap the vproj matmuls, U runs after its V is written ----
        psU0 = new_psU()
        kproj(0, 0)
        for n in range(NCH):
            if n + 1 < NCH:
                nc.sync.dma_start(xT[:, n + 1], xT_src[:, n + 1])
            for i in range(CH // 2):
                kt0 = n * CH + 2 * i
                qk1(0, 0, kt0, exp_engine(kt0))
                qk1(0, 0, kt0 + 1, exp_engine(kt0 + 1))
                vproj(kt0)
                vproj(kt0 + 1)
                if i == 0 and n + 1 < NCH:
                    kproj(0, n + 1)
            for i in range(CH // 2):
                u_pair(0, n * CH // 2 + i, psU0)
            if (n + 1) * CH == SK_T:
                finish_block(0, 0, psU0)

        # ---- remaining blocks, qc-major; fillers carry the next block's
        # projections plus the first-half output projections ----
        blocks = [(j, 0) for j in range(1, DT)]
        blocks += [(j, 1) for j in range(DT)] if NQC > 1 else []
        owed = {blk: [] for blk in blocks}
        for (j, qc) in blocks:
            if not (j == 0 and qc <= 1):
                owed[(j, qc)].append(lambda j=j, qc=qc: qproj(j, qc))
            if qc == 0 and j >= 1:
                for n in range(NCH):
                    owed[(j, qc)].append(lambda j=j, n=n: kproj(j, n))
        # first-half outproj: OT rows for qc=0 complete after block (DT-1, 0);
        # run them inside the following blocks
        if NQC > 1:
            mid_i = blocks.index((0, 1))
            later = blocks[mid_i + 1]
            owed[later].append(lambda: gate_outproj((DT - 1, 0)))
            half = SQ_T // NQC
            for qt in range(half // 2):
                owed[later].append(lambda qt=qt: outproj(qt))
            for qt in range(half // 2, half):
                owed[blocks[mid_i + 2]].append(lambda qt=qt: outproj(qt))

        qt_lo = SQ_T // NQC if NQC > 1 else 0
        open_psO = []

        def open_chain(qt):
            """Partial outproj chain (gate + first DT-1 weight tiles): its
            OT inputs are ready before the last block, so it can fill the
            last block's PE bubbles; the final tile waits the last
            normalize."""
            psO = psum.tile([P, D], F32, tag="proj", name="psO")
            restart = obias_mm(psO, qt)
            for j in range(DT - 1):
                nc.tensor.matmul(psO, OT[:, j, qt * P:(qt + 1) * P],
                                 w_bf["wo"][:, j, :],
                                 start=(restart and j == 0), stop=False)
            open_psO.append((qt, psO))

        last_fillers = [lambda: open_chain(qt_lo),
                        lambda: open_chain(qt_lo + 1)]

        for f in owed[blocks[0]]:
            f()
        for bi, (j, qc) in enumerate(blocks):
            fillers = []
            if bi + 1 < len(blocks):
                fillers += owed[blocks[bi + 1]]
            else:
                fillers += last_fillers
            psU = new_psU()
            attn_span(j, qc, list(range(SK_T)), psU, fillers,
                      drain_dve=(bi == len(blocks) - 1),
                      act_num=ACT_NUM)

        # ---- tail: last block's normalize + remaining output rows;
        # the two partially-open chains were emitted inside the last block.
        gate_outproj(blocks[-1])
        for qt, psO in open_psO:
            nc.tensor.matmul(psO, OT[:, DT - 1, qt * P:(qt + 1) * P],
                             w_bf["wo"][:, DT - 1, :],
                             start=False, stop=True)
            o_sb = work.tile([P, D], F32, tag="osb", bufs=2, name="o_sb")
            nc.scalar.activation(o_sb, psO, AF.Relu)
            nc.sync.dma_start(out[qt * P:(qt + 1) * P, :], o_sb)
        for qt in range(qt_lo + 2, SQ_T):
            outproj(qt)


_NC_CACHE = {}


def _get_nc(sk=S, sq=SQ_FULL, skip_vbias=False, skip_obias=False):
    key = (sk, sq, skip_vbias, skip_obias)
    if key not in _NC_CACHE:
        _NC_CACHE[key] = build_mha(sk, sq, skip_vbias, skip_obias)
    return _NC_CACHE[key]


def _tile_rows(a):
    """[D, n] -> SBUF layout [P, DT*n]: partition p gets rows p, 128+p, ..."""
    Dd, n = a.shape
    t = Dd // P
    return np.ascontiguousarray(
        a.reshape(t, P, n).transpose(1, 0, 2).reshape(P, t * n))


def _tile_chunks(a, chp):
    """[D, sk] -> chunk-major SBUF layout [P, NCH*DT*chp]: per partition,
    sequence chunks outermost so each chunk is one contiguous linear DMA."""
    Dd, sk = a.shape
    t, nch = Dd // P, sk // chp
    return np.ascontiguousarray(
        a.reshape(t, P, nch, chp).transpose(1, 2, 0, 3).reshape(P, -1))


def prep_inputs(x, Wq, bq, Wk, bk, Wv, bv, Wo, bo):
    """Host-side sharding/layout prep: bf16 casts, feature-major transpose,
    SBUF pre-tiling.  Returns the 8 per-core input maps."""
    bf = ml_dtypes.bfloat16
    f8 = ml_dtypes.float8_e4m3
    x = np.asarray(x, dtype=np.float32)
    shared = {
        "wq": _tile_rows(np.asarray(Wq, np.float32).astype(f8)),
        "wk": _tile_rows(np.asarray(Wk, np.float32).astype(f8)),
        "wv": _tile_rows(np.asarray(Wv, np.float32).astype(f8)),
        "wo": _tile_rows(np.asarray(Wo, np.float32).astype(bf)),
        "bq": np.ascontiguousarray(
            np.asarray(bq, np.float32).reshape(DT, P).T),
        "bk": np.ascontiguousarray(
            np.asarray(bk, np.float32).reshape(DT, P).T),
        "bv": np.asarray(bv, np.float32).astype(bf).reshape(1, D),
        "bo": np.asarray(bo, np.float32).astype(bf).reshape(1, D),
    }
    xT_b = [x[b].T.astype(f8) for b in range(B)]
    xT_tiled = [_tile_chunks(xb, 4 * P) for xb in xT_b]
    in_maps = []
    for c in range(NCORES):
        b, qo = divmod(c, QSPLIT)
        m = dict(shared)
        m["xT_f8"] = xT_tiled[b]
        m["xqT_f8"] = _tile_rows(
            xT_b[b][:, qo * SQ_FULL:(qo + 1) * SQ_FULL])
        in_maps.append(m)
    return in_maps


def kernel(x, Wq, bq, Wk, bk, Wv, bv, Wo, bo, **run_kwargs):
    """Full-input entry point: shards across 8 NeuronCores, returns full out."""
    in_maps = prep_inputs(x, Wq, bq, Wk, bk, Wv, bv, Wo, bo)
    nc = _get_nc(skip_vbias=bool(np.all(np.asarray(bv) == 0)),
                 skip_obias=bool(np.all(np.asarray(bo) == 0)))
    res = bass_utils.run_bass_kernel_spmd(
        nc, in_maps, core_ids=list(range(NCORES)), **run_kwargs)
    full = np.empty((B, S, D), np.float32)
    for c in range(NCORES):
        b, qo = divmod(c, QSPLIT)
        full[b, qo * SQ_FULL:(qo + 1) * SQ_FULL] = res.results[c]["out"]
    if run_kwargs:
        return full, res
    return full


# revision 55
# speedup vs baseline: 1.0574x; 1.0480x over previous
"""Trainium2 Bass kernel for nn_MultiHeadAttention (B=2, S=4096, D=512, H=8).

Computes: q/k/v = relu(x@W+b) per head, softmax(q k^T / sqrt(64)) v,
out = relu(concat_heads @ Wo + bo).

Sharding: 8 cores = 2 (batch) x 4 (query-slice).  Each core computes full
K/V projections for its batch (redundant across the 4 q-slice cores) and
attention + output projection for its 1024-row query slice.  No collectives;
the host concatenates the 8 output slices.

Per-core pipeline (vs the v1 baseline; ~372us -> ~308us):
- Projections (Q/K/V) run as fp8 DoubleRow matmuls (x and Wq/Wk/Wv in
  fp8e4, contracting 256 features per instruction); the output projection
  stays bf16 (fp8 there costs ~5x the final-error budget).  Bias+relu is
  fused into the PSUM->SBUF eviction (Q/V on ACT, K on DVE).
- Attention iterates ktiles: per ktile one pair of QK matmuls (the two
  heads of a pair sit in PE row groups 0-63/64-127 and run concurrently),
  one 1024-wide exp op over both heads' scores writes fp8 probabilities
  into an 8-slot SBUF ring, and (lagged 5 ktiles for pipelining) one fp8
  DoubleRow U matmul per head consumes two ring slots (contracting 256
  sequence positions per instruction).  V is stored fp8 with a ones
  column so U row 64 accumulates the softmax denominator for free.
- exp is split ~17/32 ACT (exact exp, fp8 out, bias folds the range
  shift) / 15/32 DVE (Schraudolph bit-trick: round(s*A+B) as int8 IS the
  fp8 of exp - one tensor_scalar op; ~2.7% rms which softmax
  normalization + the 2e-2 gate tolerate), so both engines compute exps
  concurrently with the PE.
- Denominator reciprocals run on ACT as exp(-ln(d)) over [1, 1024] rows
  (a patched table-set selection keeps exp/ln/relu in one ACT table so
  nothing reloads); broadcasts on gpsimd; normalize multiplies on DVE.
- Deferred projections and the first-half output projections fill PE
  bubbles inside exp-bound attention stretches; output rows overlap the
  final normalize via partially-open PSUM accumulation chains.
"""

import numpy as np
import ml_dtypes

import concourse.bass as bass
import concourse.mybir as mybir
import concourse.tile as tile
from concourse import bacc
from concourse import bass_utils
from concourse import hw_specs


def _patch_act_tables():
    """Make exp/relu/ln all resolve to the one table set that contains all
    three (natural_log_exp_and_others).  The load-insertion pass assigns
    each ACTIVATE the *first* set containing its function, so a kernel
    mixing exp and ln otherwise reloads tables around every ln (~2.7us per
    switch).  Only set *selection* changes; set contents seen by the
    runtime are untouched."""
    if getattr(hw_specs, "_mha_act_patch", False):
        return
    orig = hw_specs.get_activation_tables
    HOME = "natural_log_exp_and_others"
    AF_ = mybir.ActivationFunctionType

    def patched(arch):
        tables = orig(arch)
        if HOME not in tables:
            return tables
        out = {}
        for name, funcs in tables.items():
            if name != HOME:
                funcs = funcs - {AF_.Exp, AF_.Relu, AF_.Ln}
            out[name] = funcs
        return out

    hw_specs.get_activation_tables = patched
    bacc.get_activation_tables = patched
    hw_specs._mha_act_patch = True

F32 = mybir.dt.float32
BF16 = mybir.dt.bfloat16
FP8 = mybir.dt.float8e4
I8 = mybir.dt.int8
AF = mybir.ActivationFunctionType
ALU = mybir.AluOpType
DR = mybir.MatmulPerfMode.DoubleRow

P = 128
D = 512
H = 8
DH = 64
DT = D // P  # 4 (also = number of head pairs)
B = 2
S = 4096
NCORES = 8
QSPLIT = 4
SQ_FULL = S // QSPLIT  # 1024 query rows per core
QC = 512               # q-chunk (matmul free dim / PSUM bank width)
VP = 80                # padded V row stride (65 used; 80 keeps fp8 16B align)

# exp folding: pT = exp(s/8 + EXPB); the e^EXPB factor cancels in normalize.
EXPB = -2.9
LOG2E = 1.4426950408889634
# DVE bit-trick: int8(round(s*A8 + B8)) bits == fp8e4(exp(s/8 + EXPB))
A8 = (1 << 3) * LOG2E / 8.0
C8 = 0.35
B8 = 7 * (1 << 3) + (1 << 3) * LOG2E * EXPB - C8

# exp engine split: ACT_NUM of every 32 ktiles use ACT (exact exp);
# the rest use the DVE bit-trick.  Spread evenly (Bresenham).
ACT_NUM = 17


def exp_engine(kt, act_num=ACT_NUM):
    i = kt % 32
    return "a" if (i + 1) * act_num // 32 > i * act_num // 32 else "d"


def build_mha(sk=S, sq=SQ_FULL, skip_vbias=False, skip_obias=False):
    """Build the SPMD Bass program (identical on all cores).

    All inputs arrive pre-tiled by the host into exact SBUF layout
    ([128 partitions, contiguous free bytes]) so every load is a max-packet
    linear DMA."""
    _patch_act_tables()
    nc = bacc.Bacc("TRN2", target_bir_lowering=False, debug=False,
                   num_devices=NCORES)

    xT_d = nc.dram_tensor("xT_f8", (P, DT * sk), FP8,
                          kind="ExternalInput").ap()  # chunk-major, see prep
    xqT_d = nc.dram_tensor("xqT_f8", (P, DT * sq), FP8,
                           kind="ExternalInput").ap()
    w_dram = {}
    for n in ("wq", "wk", "wv"):
        w_dram[n] = nc.dram_tensor(n, (P, DT * D), FP8,
                                   kind="ExternalInput").ap()
    w_dram["wo"] = nc.dram_tensor("wo", (P, DT * D), BF16,
                                  kind="ExternalInput").ap()
    b_dram = {
        "bq": nc.dram_tensor("bq", (P, DT), F32, kind="ExternalInput").ap(),
        "bk": nc.dram_tensor("bk", (P, DT), F32, kind="ExternalInput").ap(),
        "bv": nc.dram_tensor("bv", (1, D), BF16, kind="ExternalInput").ap(),
        "bo": nc.dram_tensor("bo", (1, D), BF16, kind="ExternalInput").ap(),
    }
    out = nc.dram_tensor("out", (sq, D), F32, kind="ExternalOutput").ap()

    with tile.TileContext(nc) as tc:
        _build_tile(tc, xT_d, xqT_d, w_dram, b_dram, out, sk, sq,
                    skip_vbias, skip_obias)

    nc.compile()
    return nc


def _build_tile(tc, xT_d, xqT_d, w_dram, b_dram, out, sk, sq,
                skip_vbias=False, skip_obias=False):
    nc = tc.nc
    SK_T = sk // P            # ktiles of the key/value sequence (32)
    NKTP = SK_T // 2          # ktile pairs per head (16)
    SQ_T = sq // P
    NQC = sq // QC            # q chunks per core (2)
    CH = min(4, SK_T)         # stiles per projection chunk
    NCH = SK_T // CH

    with (
        tc.tile_pool(name="singles", bufs=1) as singles,
        tc.tile_pool(name="work", bufs=3) as work,
        tc.tile_pool(name="psum", bufs=2, space="PSUM") as psum,
    ):
        # ---- startup: only what Q-proj pair 0 needs, first ----
        w_bf = {}
        w_bf["wq"] = singles.tile([P, DT, D], FP8, name="wq_f8")
        wq_src = w_dram["wq"].rearrange("p (t n) -> p t n", t=DT)
        nc.sync.dma_start(w_bf["wq"][:, 0:2], wq_src[:, 0:2])
        xTq = singles.tile([P, DT, sq], FP8)
        xTq_src = xqT_d.rearrange("p (t s) -> p t s", t=DT)
        nc.scalar.dma_start(xTq[:, 0:2], xTq_src[:, 0:2])
        b_col = {}
        b_col["bq"] = singles.tile([P, DT], F32, name="bq_col")
        nc.sync.dma_start(w_bf["wq"][:, 2:4], wq_src[:, 2:4])
        nc.scalar.dma_start(xTq[:, 2:4], xTq_src[:, 2:4])
        nc.scalar.dma_start(b_col["bq"], b_dram["bq"])

        QT = singles.tile([P, DT, sq], BF16)

        def qproj(j, nq):
            psQ = psum.tile([P, QC], F32, tag="proj", name="psQ")
            for t2 in range(DT // 2):
                nc.tensor.matmul(
                    psQ, w_bf["wq"][:, 2 * t2:2 * t2 + 2, j * P:(j + 1) * P],
                    xTq[:, 2 * t2:2 * t2 + 2, nq * QC:(nq + 1) * QC],
                    start=(t2 == 0), stop=(t2 == DT // 2 - 1),
                    perf_mode=DR)
            nc.scalar.activation(
                QT[:, j, nq * QC:(nq + 1) * QC], psQ, AF.Relu,
                bias=b_col["bq"][:, j:j + 1])

        qproj(0, 0)
        if NQC > 1:
            qproj(0, 1)

        # ---- K-proj deps next (attention can start before V exists) ----
        b_row = {}
        w_bf["wk"] = singles.tile([P, DT, D], FP8, name="wk_f8")
        nc.scalar.dma_start(w_bf["wk"], w_dram["wk"].rearrange(
            "p (t n) -> p t n", t=DT))
        b_col["bk"] = singles.tile([P, DT], F32, name="bk_col")
        nc.scalar.dma_start(b_col["bk"], b_dram["bk"])
        CHP = CH * P
        xT = singles.tile([P, NCH, DT, CHP], FP8)
        xT_src = xT_d.rearrange("p (n t s) -> p n t s", n=NCH, t=DT)
        nc.sync.dma_start(xT[:, 0], xT_src[:, 0])
        for n in ("wv", "wo"):
            dt_n = BF16 if n == "wo" else FP8
            wb = singles.tile([P, DT, D], dt_n, name=f"{n}_w")
            nc.sync.dma_start(wb, w_dram[n].rearrange(
                "p (t n) -> p t n", t=DT))
            w_bf[n] = wb
            if n == "wv" and not skip_vbias:
                br = singles.tile([1, D], BF16, name="bv_row")
                nc.sync.dma_start(br, b_dram["bv"])
                b_row["bv"] = br
        br = singles.tile([1, D], BF16, name="bo_row")
        nc.sync.dma_start(br, b_dram["bo"])
        b_row["bo"] = br

        # ---- persistent SBUF tensors ----
        bias_t = singles.tile([P, 1], F32)
        nc.vector.memset(bias_t, EXPB)
        xT1 = None
        if not skip_vbias:
            xT1 = singles.tile([1, sk], BF16)
            nc.vector.memset(xT1, 1.0)
        KT = singles.tile([P, DT, sk], BF16)
        V_pad = singles.tile([P, NKTP, H, 2, VP], FP8)
        nc.vector.memset(V_pad[:, :, :, :, DH:DH + 1], 1.0)
        OT = singles.tile([P, DT, sq], BF16)
        OT1 = singles.tile([1, sq], BF16)
        nc.vector.memset(OT1, 1.0)

        # PSUM tags: "proj" 2x1 banks, "scores" 2x2 banks, "psU" 1x2 = 8
        def vproj(st):
            n, si = st // CH, st % CH
            psV = psum.tile([P, D], F32, tag="proj", name="psV")
            for t2 in range(DT // 2):
                nc.tensor.matmul(
                    psV, xT[:, n, 2 * t2:2 * t2 + 2, si * P:(si + 1) * P],
                    w_bf["wv"][:, 2 * t2:2 * t2 + 2, :],
                    start=(t2 == 0),
                    stop=(skip_vbias and t2 == DT // 2 - 1),
                    perf_mode=DR)
            if not skip_vbias:
                nc.tensor.matmul(psV, xT1[:, st * P:(st + 1) * P],
                                 b_row["bv"], start=False, stop=True)
            nc.scalar.activation(
                V_pad[:, st // 2, :, st % 2, 0:DH],
                psV.rearrange("p (h d) -> p h d", h=H), AF.Relu)

        def kproj(j, n):
            psK = psum.tile([P, CH * P], F32, tag="proj", name="psK")
            for t2 in range(DT // 2):
                nc.tensor.matmul(
                    psK, w_bf["wk"][:, 2 * t2:2 * t2 + 2, j * P:(j + 1) * P],
                    xT[:, n, 2 * t2:2 * t2 + 2, :],
                    start=(t2 == 0), stop=(t2 == DT // 2 - 1),
                    perf_mode=DR)
            nc.vector.tensor_scalar(
                KT[:, j, n * CH * P:(n + 1) * CH * P], psK,
                b_col["bk"][:, j:j + 1], 0.0, op0=ALU.add, op1=ALU.max)

        # fp8 probability ring: slot kt%RING holds exp'd scores for both
        # heads of one ktile; the U matmul reads two adjacent slots with a
        # strided DoubleRow access pattern.
        RING = 8
        PT = singles.tile([P, RING, 2, QC], FP8, name="PT_ring")
        # prefetch buffers: a qc=1 block's probabilities are computed
        # ahead of time (block (0,1) during the chunk phase, each later
        # qc=1 block during its predecessor) where engines have slack;
        # the consuming block then runs U-matmuls only, with no
        # score-buffer turnover of its own.
        PT2 = singles.tile([P, SK_T, 2, QC], FP8, name="PT2_pre")
        PT3 = singles.tile([P, SK_T, 2, QC], FP8, name="PT3_pre")

        def qk1(j, qc, kt, eng, ring=None, slot=None):
            """Scores + exp for BOTH heads of pair j at ktile kt.  The two
            QK matmuls sit in different PE row groups (partitions 0-63 vs
            64-127) and run concurrently; one 1024-wide exp op (eng 'a' =
            ACT exact exp->fp8, 'd' = DVE bit-trick int8-as-fp8) covers
            both heads."""
            q0 = qc * QC
            psS = psum.tile([P, 2, QC], F32, tag="scores", bufs=2,
                            name="psS")
            for a in (0, 1):
                h0 = a * DH
                nc.tensor.matmul(
                    psS[:, a, :],
                    KT[h0:h0 + DH, j, kt * P:(kt + 1) * P],
                    QT[h0:h0 + DH, j, q0:q0 + QC], start=True, stop=True)
            if ring is None:
                ring, slot = PT, kt % RING
            pT_f = ring[:, slot].rearrange("p a b -> p (a b)")
            psS_f = psS.rearrange("p a b -> p (a b)")
            if eng == "a":
                nc.scalar.activation(pT_f, psS_f, AF.Exp, scale=0.125,
                                     bias=bias_t)
            else:
                nc.vector.tensor_scalar(pT_f.bitcast(I8), psS_f, A8, B8,
                                        op0=ALU.mult, op1=ALU.add)

        def u_pair(j, tp, psU, ring=None, s0=None):
            """DoubleRow U matmuls for both heads of ktile pair tp, reading
            ring slots (2tp)%RING, (2tp)%RING+1 (slot stride 2*QC fp8)."""
            if ring is None:
                ring, s0 = PT, (2 * tp) % RING
            for a in (0, 1):
                nc.tensor.matmul(
                    psU[:, a, :], V_pad[:, tp, 2 * j + a, :, 0:DH + 1],
                    ring[:, s0:s0 + 2, a, :],
                    start=(tp == 0), stop=(tp == NKTP - 1), perf_mode=DR)

        brc_sink = {}

        def finish_block(j, qc, psU):
            """U done for both heads: copy U rows out of PSUM, compute
            1/denominator on ACT (exp(-ln d)), then normalize on gpsimd."""
            q0 = qc * QC
            ucs = work.tile([DH, 2, QC], F32, tag="ucopy", bufs=2,
                            name="ucs")
            nc.vector.tensor_copy(ucs, psU[0:DH])
            # Ln reads the denominator row at partition 64 and lands it at
            # partition 0 (ACT maps partitions relative to the AP base)
            lnd = work.tile([1, 2 * QC], F32, tag="lnd", bufs=1, name="lnd")
            nc.scalar.activation(
                lnd, psU[DH:DH + 1].rearrange("p a b -> p (a b)"), AF.Ln)
            rcp = work.tile([1, 2 * QC], F32, tag="rcp", bufs=1, name="rcp")
            nc.scalar.activation(rcp, lnd, AF.Exp, scale=-1.0)
            for a in (0, 1):
                h0 = a * DH
                brc = work.tile([DH, QC], F32, tag="brc", bufs=4,
                                name="brc")
                nc.gpsimd.partition_broadcast(
                    brc, rcp[0:1, a * QC:a * QC + QC])
                nc.vector.tensor_mul(
                    OT[h0:h0 + DH, j, q0:q0 + QC], ucs[:, a, :], brc)
                brc_sink[(j, qc)] = brc

        def attn_span(j, qc, kts, psU, fillers=(), drain_dve=False,
                      act_num=ACT_NUM):
            """Emit one attention block: per ktile a QK pair + exp
            (engines alternating by ktile), with the U matmul pair lagging
            two ktiles behind so the in-order PE never waits on an exp.
            Fillers (deferred projections) slot in between ktiles."""
            fillers = list(fillers)
            spacing = max(1, (3 * len(kts) // 4) // (len(fillers) + 1))
            for i, kt in enumerate(kts):
                if kt >= 5 and kt % 2 == 1:
                    u_pair(j, (kt - 5) // 2, psU)
                eng = "a" if (drain_dve and kt >= SK_T - 6) \
                    else exp_engine(kt, act_num)
                qk1(j, qc, kt, eng)
                if fillers and (i + 1) % spacing == 0:
                    fillers.pop(0)()
            for f in fillers:
                f()
            if kts[-1] == SK_T - 1:
                u_pair(j, NKTP - 2, psU)
                u_pair(j, NKTP - 1, psU)
                finish_block(j, qc, psU)

        def new_psU():
            return psum.tile([DH + 1, 2, QC], F32, tag="psU", bufs=1,
                             name="psU")

        def obias_mm(psO, qt):
            """Ordering gate (+ bias when bo!=0): reads OT1, whose re-write
            after the last normalize gates the chain.  With bo==0 only the
            dependency matters, so stream a single column (~60 cycles); the
            j=0 weight matmul then clears PSUM with start=True."""
            if skip_obias:
                nc.tensor.matmul(psO[:, 0:1], OT1[:, qt * P:(qt + 1) * P],
                                 b_row["bo"][:, 0:1], start=True, stop=False,
                                 skip_group_check=True)
                return True
            nc.tensor.matmul(psO, OT1[:, qt * P:(qt + 1) * P],
                             b_row["bo"], start=True, stop=False)
            return False

        def outproj(qt):
            psO = psum.tile([P, D], F32, tag="proj", name="psO")
            restart = obias_mm(psO, qt)
            for j in range(DT):
                nc.tensor.matmul(psO, OT[:, j, qt * P:(qt + 1) * P],
                                 w_bf["wo"][:, j, :],
                                 start=(restart and j == 0),
                                 stop=(j == DT - 1))
            o_sb = work.tile([P, D], F32, tag="osb", bufs=2, name="o_sb")
            nc.scalar.activation(o_sb, psO, AF.Relu)
            nc.sync.dma_start(out[qt * P:(qt + 1) * P, :], o_sb)

        def gate_outproj(blk):
            """No-op rewrite of OT1 (max(1, rcp<1) == 1) that depends on
            block `blk`'s normalize chain — gates the outproj chains (which
            start with an OT1-reading bias matmul) behind it."""
            brc = brc_sink[blk]
            nc.vector.tensor_scalar(OT1, OT1, brc[0:1, 0:1], None,
                                    op0=ALU.max)

        # ---- chunk loop: x load + V proj + K proj(pair 0) + attn(0, 0);
        # exps overlap the vproj matmuls, U runs after its V is written ----
        psU0 = new_psU()
        kproj(0, 0)
        for n in range(NCH):
            if n + 1 < NCH:
                nc.sync.dma_start(xT[:, n + 1], xT_src[:, n + 1])
            for i in range(CH // 2):
                kt0 = n * CH + 2 * i
                qk1(0, 0, kt0, exp_engine(kt0))
                qk1(0, 0, kt0 + 1, exp_engine(kt0 + 1))
                vproj(kt0)
                vproj(kt0 + 1)
                if i == 0 and n + 1 < NCH:
                    kproj(0, n + 1)
                flip = {"a": "d", "d": "a"}
                qk1(0, 1, kt0, flip[exp_engine(kt0)], PT2, kt0)
                qk1(0, 1, kt0 + 1, flip[exp_engine(kt0 + 1)], PT2, kt0 + 1)
            for i in range(CH // 2):
                u_pair(0, n * CH // 2 + i, psU0)
            if (n + 1) * CH == SK_T:
                finish_block(0, 0, psU0)

        # ---- remaining blocks, qc-major; fillers carry the next block's
        # projections plus the first-half output projections ----
        blocks = [(j, 0) for j in range(1, DT)]
        blocks += [(j, 1) for j in range(DT)] if NQC > 1 else []
        owed = {blk: [] for blk in blocks}
        for (j, qc) in blocks:
            if not (j == 0 and qc <= 1):
                owed[(j, qc)].append(lambda j=j, qc=qc: qproj(j, qc))
            if qc == 0 and j >= 1:
                for n in range(NCH):
                    owed[(j, qc)].append(lambda j=j, n=n: kproj(j, n))
        # first-half outproj: OT rows for qc=0 complete after block (DT-1, 0);
        # run them inside the following blocks
        if NQC > 1:
            mid_i = blocks.index((0, 1))
            later = blocks[mid_i + 1]
            owed[later].append(lambda: gate_outproj((DT - 1, 0)))
            half = SQ_T // NQC
            for qt in range(half // 2):
                owed[later].append(lambda qt=qt: outproj(qt))
            for qt in range(half // 2, half):
                owed[blocks[mid_i + 2]].append(lambda qt=qt: outproj(qt))

        qt_lo = SQ_T // NQC if NQC > 1 else 0
        open_psO = []

        def open_chain(qt):
            """Partial outproj chain (gate + first DT-1 weight tiles): its
            OT inputs are ready before the last block, so it can fill the
            last block's PE bubbles; the final tile waits the last
            normalize."""
            psO = psum.tile([P, D], F32, tag="proj", name="psO")
            restart = obias_mm(psO, qt)
            for j in range(DT - 1):
                nc.tensor.matmul(psO, OT[:, j, qt * P:(qt + 1) * P],
                                 w_bf["wo"][:, j, :],
                                 start=(restart and j == 0), stop=False)
            open_psO.append((qt, psO))

        last_fillers = [lambda: open_chain(qt_lo),
                        lambda: open_chain(qt_lo + 1)]

        pre_bufs = {(0, 1): PT2}

        for f in owed[blocks[0]]:
            f()
        for bi, (j, qc) in enumerate(blocks):
            fillers = []
            if bi + 1 < len(blocks):
                fillers += owed[blocks[bi + 1]]
            else:
                fillers += last_fillers
            psU = new_psU()
            if qc == 1:
                cur = pre_bufs[(j, qc)]
                nxt_blk = (blocks[bi + 1]
                           if bi + 1 < len(blocks)
                           and blocks[bi + 1][1] == 1 else None)
                nxt_buf = None
                fillers = list(fillers)
                if nxt_blk is not None:
                    nxt_buf = PT3 if cur is PT2 else PT2
                    pre_bufs[nxt_blk] = nxt_buf
                    # the next block's qproj must precede its prefetched QKs
                    fillers.pop(0)()
                spacing = max(1, (3 * NKTP // 4) // (len(fillers) + 1))
                for tp in range(NKTP):
                    u_pair(j, tp, psU, cur, 2 * tp)
                    if nxt_blk is not None:
                        for dk in (0, 1):
                            kt = 2 * tp + dk
                            qk1(nxt_blk[0], 1, kt, exp_engine(kt),
                                nxt_buf, kt)
                    if fillers and (tp + 1) % spacing == 0:
                        fillers.pop(0)()
                for f in fillers:
                    f()
                finish_block(j, qc, psU)
            else:
                attn_span(j, qc, list(range(SK_T)), psU, fillers,
                          drain_dve=False, act_num=ACT_NUM)

        # ---- tail: last block's normalize + remaining output rows;
        # the two partially-open chains were emitted inside the last block.
        gate_outproj(blocks[-1])
        for qt, psO in open_psO:
            nc.tensor.matmul(psO, OT[:, DT - 1, qt * P:(qt + 1) * P],
                             w_bf["wo"][:, DT - 1, :],
                             start=False, stop=True)
            o_sb = work.tile([P, D], F32, tag="osb", bufs=2, name="o_sb")
            nc.scalar.activation(o_sb, psO, AF.Relu)
            nc.sync.dma_start(out[qt * P:(qt + 1) * P, :], o_sb)
        for qt in range(qt_lo + 2, SQ_T):
            outproj(qt)


_NC_CACHE = {}


def _get_nc(sk=S, sq=SQ_FULL, skip_vbias=False, skip_obias=False):
    key = (sk, sq, skip_vbias, skip_obias)
    if key not in _NC_CACHE:
        _NC_CACHE[key] = build_mha(sk, sq, skip_vbias, skip_obias)
    return _NC_CACHE[key]


def _tile_rows(a):
    """[D, n] -> SBUF layout [P, DT*n]: partition p gets rows p, 128+p, ..."""
    Dd, n = a.shape
    t = Dd // P
    return np.ascontiguousarray(
        a.reshape(t, P, n).transpose(1, 0, 2).reshape(P, t * n))


def _tile_chunks(a, chp):
    """[D, sk] -> chunk-major SBUF layout [P, NCH*DT*chp]: per partition,
    sequence chunks outermost so each chunk is one contiguous linear DMA."""
    Dd, sk = a.shape
    t, nch = Dd // P, sk // chp
    return np.ascontiguousarray(
        a.reshape(t, P, nch, chp).transpose(1, 2, 0, 3).reshape(P, -1))


def prep_inputs(x, Wq, bq, Wk, bk, Wv, bv, Wo, bo):
    """Host-side sharding/layout prep: bf16 casts, feature-major transpose,
    SBUF pre-tiling.  Returns the 8 per-core input maps."""
    bf = ml_dtypes.bfloat16
    f8 = ml_dtypes.float8_e4m3
    x = np.asarray(x, dtype=np.float32)
    shared = {
        "wq": _tile_rows(np.asarray(Wq, np.float32).astype(f8)),
        "wk": _tile_rows(np.asarray(Wk, np.float32).astype(f8)),
        "wv": _tile_rows(np.asarray(Wv, np.float32).astype(f8)),
        "wo": _tile_rows(np.asarray(Wo, np.float32).astype(bf)),
        "bq": np.ascontiguousarray(
            np.asarray(bq, np.float32).reshape(DT, P).T),
        "bk": np.ascontiguousarray(
            np.asarray(bk, np.float32).reshape(DT, P).T),
        "bv": np.asarray(bv, np.float32).astype(bf).reshape(1, D),
        "bo": np.asarray(bo, np.float32).astype(bf).reshape(1, D),
    }
    xT_b = [x[b].T.astype(f8) for b in range(B)]
    xT_tiled = [_tile_chunks(xb, 4 * P) for xb in xT_b]
    in_maps = []
    for c in range(NCORES):
        b, qo = divmod(c, QSPLIT)
        m = dict(shared)
        m["xT_f8"] = xT_tiled[b]
        m["xqT_f8"] = _tile_rows(
            xT_b[b][:, qo * SQ_FULL:(qo + 1) * SQ_FULL])
        in_maps.append(m)
    return in_maps


def kernel(x, Wq, bq, Wk, bk, Wv, bv, Wo, bo, **run_kwargs):
    """Full-input entry point: shards across 8 NeuronCores, returns full out."""
    in_maps = prep_inputs(x, Wq, bq, Wk, bk, Wv, bv, Wo, bo)
    nc = _get_nc(skip_vbias=bool(np.all(np.asarray(bv) == 0)),
                 skip_obias=bool(np.all(np.asarray(bo) == 0)))
    res = bass_utils.run_bass_kernel_spmd(
        nc, in_maps, core_ids=list(range(NCORES)), **run_kwargs)
    full = np.empty((B, S, D), np.float32)
    for c in range(NCORES):
        b, qo = divmod(c, QSPLIT)
        full[b, qo * SQ_FULL:(qo + 1) * SQ_FULL] = res.results[c]["out"]
    if run_kwargs:
        return full, res
    return full
